# revision 19
# baseline (speedup 1.0000x reference)
"""Grouped linear (MoE routing) kernel for 8 Trainium2 NeuronCores.

out[n] = x[n] @ weight[g[n]].T + bias[g[n]]

Strategy: expert-parallel. group_indices is (assumed) sorted; host code
computes per-group row ranges, pads each group's rows to a common
capacity C (multiple of 128), and core g computes the dense GEMM
  out_g = x_g @ weight[g].T + bias[g]
entirely on-core with no collectives. Host gathers/scatters rows.

Per-core Bass kernel (SPMD, identical program on 8 cores):
  - x and W are cast to bf16 on the host. The PE runs bf16 at the same
    1 column/cycle as fp32r, so the compute floor is unchanged, but W
    drops to 8 MB and x to 0.5 MB/tile: the startup W load fits well
    under phase-A compute and steady-state HBM traffic halves. fp32
    accumulation in PSUM + fp32 bias/output keep rel err ~3e-3, well
    inside the 2e-2 gate.
  - W^T stays resident in SBUF, DMA'd in per-(n, ko) 128 KB chunks so
    the first matmul waits only ~2 us and the PE tracks delivery.
  - PSUM [128, 512] accumulates over the 16 k-subtiles; bias add happens
    on the VectorE during PSUM->SBUF eviction.
"""

import math
import sys

for _p in ("/opt/trn_rl_repo", "/root/.axon_site/_ro/trn_rl_repo"):
    if _p not in sys.path:
        sys.path.append(_p)

import ml_dtypes
import numpy as np

BF16 = np.dtype(ml_dtypes.bfloat16)

from concourse import bacc, mybir, tile
from concourse.bass_utils import run_bass_kernel_spmd

P = 128
D_IN = 2048
D_OUT = 2048
KO = D_IN // P  # 16 k-subtiles
N_TILE = 512
N_TILES = D_OUT // N_TILE  # 4
NUM_GROUPS = 8
N_CORES = 8

_nc_cache: dict = {}


def build_program(C: int, repeat: int = 1, inner: str = "n", ph_a: int = 4):
    """Build + compile the per-core Bass program for row capacity C.

    Startup scheduling: W is DMA'd in per-(n, ko) chunks (256 KB each)
    interleaved with the phase-A x tiles so the PE's first matmul only
    waits for one x half-tile + one W chunk (~2.5 us) instead of a full
    4 MB W quarter. Phase A runs n-outer over ph_a resident x tiles,
    consuming W chunks roughly at delivery rate; phase B streams the
    remaining m-tiles with W fully resident.
    """
    key = (C, repeat, inner, ph_a)
    if key in _nc_cache:
        return _nc_cache[key]
    assert C % P == 0
    m_tiles = C // P
    f32 = mybir.dt.float32
    bf16 = mybir.dt.bfloat16

    nc = bacc.Bacc(
        "TRN2", target_bir_lowering=False, debug=False, num_devices=N_CORES
    )
    # Blocked HBM layouts (prepared host-side) so every DMA moves large
    # contiguous per-partition runs:
    #   xT[m, kp, ko, j]  = x[m*128+j, ko*128+kp]   (4 KB/partition/DMA)
    #   wT[n, kp, ko, nn] = W^T[ko*128+kp, n*512+nn] (1 KB/partition/chunk)
    xT = nc.dram_tensor(
        "xT", [m_tiles, P, KO, P], bf16, kind="ExternalInput"
    ).ap()
    wT = nc.dram_tensor(
        "wT", [N_TILES, P, KO, N_TILE], bf16, kind="ExternalInput"
    ).ap()
    bb = nc.dram_tensor("bb", [P, D_OUT], f32, kind="ExternalInput").ap()
    out = nc.dram_tensor("out", [C, D_OUT], f32, kind="ExternalOutput").ap()

    ph_a = min(ph_a, m_tiles)

    with tile.TileContext(nc) as tc:
        with (
            tc.tile_pool(name="wpool", bufs=1) as wpool,
            tc.tile_pool(name="cpool", bufs=1) as cpool,
            tc.tile_pool(name="xapool", bufs=1) as xapool,
            tc.tile_pool(name="xpool", bufs=3) as xpool,
            tc.tile_pool(name="opool", bufs=4) as opool,
            tc.tile_pool(name="ofpool", bufs=2) as ofpool,
            tc.tile_pool(name="pspool", bufs=8, space="PSUM") as pspool,
        ):
            w_sb = wpool.tile([P, N_TILES, KO, N_TILE], bf16)
            b_sb = cpool.tile([P, D_OUT], f32)
            xa_sb = xapool.tile([P, ph_a, KO, P], bf16)

            KQ = 4  # ko's per W chunk: 1.46 us transfer >= 625 ns HWDGE
            n_kq = KO // KQ

            def w_chunk(n, kq):
                nc.sync.dma_start(
                    w_sb[:, n, kq * KQ : (kq + 1) * KQ],
                    wT[n, :, kq * KQ : (kq + 1) * KQ],
                )

            def xa_dma(m):
                nc.sync.dma_start(xa_sb[:, m], xT[m])

            # DMA issue order ~= HBM service order. Two phase-A x tiles
            # lead, then the n=0 W ko-quads stream; the PE consumes them
            # ko-major across the resident m-tiles so each chunk unlocks
            # several matmuls and the in-order PE stream tracks delivery.
            # Chunks are sized so the 625 ns/DMA HWDGE descriptor-gen
            # serialization stays hidden under the transfers.
            # lead-in: halve the first x tile and first W chunk so the
            # first matmul's dependency chain is ~2x shorter (each piece
            # still >= the 625 ns HWDGE descriptor-gen time).
            nc.sync.dma_start(xa_sb[:, 0, : KO // 2], xT[0, :, : KO // 2])
            nc.sync.dma_start(w_sb[:, 0, :2], wT[0, :, :2])
            nc.sync.dma_start(xa_sb[:, 0, KO // 2 :], xT[0, :, KO // 2 :])
            nc.sync.dma_start(w_sb[:, 0, 2:4], wT[0, :, 2:4])
            if ph_a > 1:
                xa_dma(1)
            for kq in range(1, n_kq):
                w_chunk(0, kq)
            for m in range(2, ph_a):
                xa_dma(m)
            nc.sync.dma_start(b_sb[:], bb[:])
            for n in range(1, N_TILES):
                for kq in range(n_kq):
                    w_chunk(n, kq)

            def evict(ps, m, n):
                ms = slice(m * P, (m + 1) * P)
                ns = slice(n * N_TILE, (n + 1) * N_TILE)
                o_sb = opool.tile([P, N_TILE], f32, tag="o")
                nc.vector.tensor_add(o_sb[:], ps, b_sb[:, ns])
                # out DMAs ride the Activation HWDGE queue so their
                # descriptor generation doesn't serialize behind the
                # x/W loads on the SP queue
                nc.scalar.dma_start(out[ms, ns], o_sb[:])

            def do_group(x_tile, m, n, o_full=None):
                ps = pspool.tile([P, N_TILE], f32, tag="ps")
                for ko in range(KO):
                    nc.tensor.matmul(
                        ps,
                        x_tile[:, ko],
                        w_sb[:, n, ko],
                        start=(ko == 0),
                        stop=(ko == KO - 1),
                    )
                if o_full is None:
                    evict(ps, m, n)
                else:
                    ns = slice(n * N_TILE, (n + 1) * N_TILE)
                    nc.vector.tensor_add(o_full[:, ns], ps, b_sb[:, ns])

            def do_mtile_ko_outer(x_tile, m):
                pss = []
                for _i in range(N_TILES):
                    ps_i = pspool.tile([P, N_TILE], f32, tag="ps", name=f"ps_{m}_{_i}")
                    pss.append(ps_i)
                for ko in range(KO):
                    for n in range(N_TILES):
                        nc.tensor.matmul(
                            pss[n],
                            x_tile[:, ko],
                            w_sb[:, n, ko],
                            start=(ko == 0),
                            stop=(ko == KO - 1),
                        )
                for n in range(N_TILES):
                    evict(pss[n], m, n)

            def phase_a_quarter(n, m_set):
                pss = {}
                for m in m_set:
                    pss[m] = pspool.tile(
                        [P, N_TILE], f32, tag="ps", name=f"psA_{n}_{m}"
                    )
                for ko in range(KO):
                    for m in m_set:
                        nc.tensor.matmul(
                            pss[m],
                            xa_sb[:, m, ko],
                            w_sb[:, n, ko],
                            start=(ko == 0),
                            stop=(ko == KO - 1),
                        )
                for m in m_set:
                    evict(pss[m], m, n)

            for rep in range(repeat):
                if rep == 0:
                    # phase A: ko-major across resident x tiles per quarter.
                    # n=0 runs in two waves because only two x tiles have
                    # arrived when its chunks start landing.
                    first = list(range(min(2, ph_a)))
                    rest = list(range(min(2, ph_a), ph_a))
                    phase_a_quarter(0, first)
                    if rest:
                        phase_a_quarter(0, rest)
                    for n in range(1, N_TILES):
                        phase_a_quarter(n, list(range(ph_a)))
                    b_start = ph_a
                else:
                    b_start = 0
                # phase B: steady-state streaming; full-row out tiles so the
                # out DMA writes 8 KB/partition contiguous
                for m in range(b_start, m_tiles):
                    x_sb = xpool.tile([P, KO, P], bf16, tag="x")
                    nc.sync.dma_start(x_sb[:], xT[m])
                    if inner == "ko":
                        do_mtile_ko_outer(x_sb, m)
                    elif m == m_tiles - 1:
                        # last tile: per-slice eviction so the final out DMA
                        # doesn't serialize behind all 4 bias-adds
                        for n in range(N_TILES):
                            do_group(x_sb, m, n)
                    else:
                        o_full = ofpool.tile([P, D_OUT], f32, tag="of")
                        for n in range(N_TILES):
                            do_group(x_sb, m, n, o_full=o_full)
                        nc.scalar.dma_start(
                            out[m * P : (m + 1) * P, :], o_full[:]
                        )

    nc.compile()
    _nc_cache[key] = nc
    return nc


def shard_inputs(x, weight, bias, group_indices):
    """Host-side expert-parallel sharding. Returns (in_maps, perm, offsets,
    counts, C)."""
    n_rows = x.shape[0]
    gi = np.asarray(group_indices)
    # Sorted in the reference's setup; stable argsort keeps it general and
    # is nearly free when already sorted.
    perm = np.argsort(gi, kind="stable")
    counts = np.bincount(gi, minlength=NUM_GROUPS).astype(np.int64)
    offsets = np.zeros(NUM_GROUPS + 1, dtype=np.int64)
    np.cumsum(counts, out=offsets[1:])
    C = max(P, int(math.ceil(counts.max() / P)) * P)

    x_sorted = x[perm] if not np.array_equal(perm, np.arange(n_rows)) else x
    m_tiles = C // P
    in_maps = []
    for g in range(NUM_GROUPS):
        ng = int(counts[g])
        xg = np.zeros((C, D_IN), dtype=np.float32)
        xg[:ng] = x_sorted[offsets[g] : offsets[g] + ng]
        # blocked layouts — see build_program
        xb = np.ascontiguousarray(
            xg.reshape(m_tiles, P, KO, P).transpose(0, 3, 2, 1).astype(BF16)
        )
        wb = np.ascontiguousarray(
            weight[g]
            .T.reshape(KO, P, N_TILES, N_TILE)
            .transpose(2, 1, 0, 3)
            .astype(BF16)
        )
        in_maps.append(
            {
                "xT": xb,
                "wT": wb,
                "bb": np.ascontiguousarray(
                    np.broadcast_to(bias[g], (P, D_OUT))
                ),
            }
        )
    return in_maps, perm, offsets, counts, C


def unshard_output(results, perm, offsets, counts, n_rows):
    out = np.empty((n_rows, D_OUT), dtype=np.float32)
    for g in range(NUM_GROUPS):
        ng = int(counts[g])
        out[perm[offsets[g] : offsets[g] + ng]] = results[g]["out"][:ng]
    return out


def kernel(x, weight, bias, group_indices):
    x = np.asarray(x, dtype=np.float32)
    weight = np.asarray(weight, dtype=np.float32)
    bias = np.asarray(bias, dtype=np.float32)
    group_indices = np.asarray(group_indices)
    assert x.shape[1] == D_IN and weight.shape == (NUM_GROUPS, D_OUT, D_IN)

    in_maps, perm, offsets, counts, C = shard_inputs(
        x, weight, bias, group_indices
    )
    nc = build_program(C)
    res = run_bass_kernel_spmd(nc, in_maps, core_ids=list(range(N_CORES)))
    return unshard_output(res.results, perm, offsets, counts, x.shape[0])


def _sim_main():
    import tsim

    C = 2176
    for ph_a in (4, 5, 6):
        _nc_cache.clear()
        nc = build_program(C, repeat=1, ph_a=ph_a)
        t1, _ = tsim.simulate(nc)
        nc3 = build_program(C, repeat=3, ph_a=ph_a)
        t3, _ = tsim.simulate(nc3)
        body = (t3 - t1) / 2
        print(
            f"ph_a={ph_a}: full {t1 / 1e3:.1f} us, body {body / 1e3:.1f} us,"
            f" overhead {(t1 - body) / 1e3:.1f} us"
        )


if __name__ == "__main__":
    _sim_main()

